# revision 18
# baseline (speedup 1.0000x reference)
"""Trainium2 Bass kernel for nn_PolymorphicSNN.

Reference math:
    h        = x @ W_lin.T + b_lin                       # [B, N]
    reg_out  = heaviside(h - 1.0)            in {0, 1}   # [B, N]
    scores   = softmax(einsum(reg_out, W_sel) + b_sel)   # [B, P, M] (finite)
    spk_mode = heaviside(reg_out - 1.0)                  # == 0 exactly!
    mixed    = M * spk_mode * scores                     # == 0 exactly
    out      = concat([reg_out, mixed.reshape(B, -1)])   # [B, N*(1+P)]

Since reg_out in {0.0, 1.0}, (reg_out - 1.0 > 0) is identically False, so
spk_mode == 0 and the whole mixed block is exactly zero (scores are finite
softmax outputs, so 0 * scores == 0). The only device work needed is the
[B, N] spike map:  reg_out = (x @ W_lin.T > 1 - b_lin).

Strategy: pure data-parallel over 8 NeuronCores, batch-sharded (1024 rows
per core). Host pre-transposes the shard (xT [N, BS]) and the weight
(WT = W_lin.T) so the contraction dim lands on SBUF partitions; the bias
and threshold fold into a single per-partition compare:
    outT[j, b] = (W @ x^T)[j, b] > (1 - b_lin[j])
as fp32 PE matmuls + DVE is_gt (uint8 out). Input DMAs are split across
both HWDGE rings (sync + scalar) and by batch half so matmuls start as
soon as the first half lands; scratch warmup matmuls keep the PE HAM
clock-gate warm through the DMA window. Host assembles the full
[8192, 8448] float32 output with the zero block.
"""

import numpy as np

import concourse.bacc as bacc
import concourse.mybir as mybir
from concourse.bass_utils import run_bass_kernel_spmd
from concourse.tile import TileContext

B_FULL = 8192
N = 256                     # features / neurons / modes
P_SEL = 32                  # polymorphic neuron count (dead on device)
NCORES = 8
BS = B_FULL // NCORES       # 1024 batch rows per core
OUT_COLS = N * (1 + P_SEL)  # 8448
NB = 512                    # matmul moving free dim (one f32 PSUM bank)
N_WARMUP = 4                # scratch matmuls to warm the PE clock gate
MEMSET_WARMUP = True        # zero the warmup scratch first (vs garbage)

_NC_CACHE = []


def _build_nc():
    nc = bacc.Bacc("TRN2", target_bir_lowering=False, debug=False)
    xT = nc.declare_dram_parameter("xT", [N, BS], mybir.dt.float32, isOutput=False)
    WT = nc.declare_dram_parameter("WT", [N, N], mybir.dt.float32, isOutput=False)
    thr = nc.declare_dram_parameter("thr", [2, 128, 1], mybir.dt.float32, isOutput=False)
    outT = nc.declare_dram_parameter("outT", [N, BS], mybir.dt.uint8, isOutput=True)

    KC = N // 128    # contraction chunks
    JC = N // 128    # output-neuron chunks
    BC = BS // NB    # batch chunks

    rings = [nc.sync, nc.scalar]  # the two HWDGE rings

    with TileContext(nc) as tc:
        with (
            tc.tile_pool(name="sbuf", bufs=1) as sbuf,
            tc.tile_pool(name="psum", bufs=1, space="PSUM") as psum,
        ):
            # --- input DMAs, split across both HWDGE rings -------------
            wt_tiles = []
            for k in range(KC):
                wt = sbuf.tile([128, N], mybir.dt.float32, tag=f"wt{k}")
                rings[k % 2].dma_start(out=wt[:], in_=WT[k * 128:(k + 1) * 128, :])
                wt_tiles.append(wt)
            thr_t = sbuf.tile([128, 2], mybir.dt.float32, tag="thr")
            nc.sync.dma_start(out=thr_t[:], in_=thr.rearrange("c p o -> p (c o)"))
            # xT [k chunk][b half] so compute can start after the b0 half
            xt_tiles = [[None] * BC for _ in range(KC)]
            for b in range(BC):
                for k in range(KC):
                    xt = sbuf.tile([128, NB], mybir.dt.float32, tag=f"xt{k}{b}")
                    rings[k % 2].dma_start(
                        out=xt[:],
                        in_=xT[k * 128:(k + 1) * 128, b * NB:(b + 1) * NB],
                    )
                    xt_tiles[k][b] = xt

            # --- PE warmup on zero scratch while inputs stream ---------
            # no deps on the DMA'd tiles, so these run during the DMA window
            wsrc = sbuf.tile([128, NB], mybir.dt.float32, tag="wsrc")
            if MEMSET_WARMUP:
                nc.vector.memset(wsrc[:], 0.0)
            wacc = psum.tile([128, NB], mybir.dt.float32, tag="wacc", name="wacc")
            for _ in range(N_WARMUP):
                nc.tensor.matmul(
                    wacc,
                    wsrc[:, 0:128],
                    wsrc[:],
                    start=True,
                    stop=True,
                    skip_group_check=True,
                )

            # --- matmul + threshold + store, b-half at a time ----------
            # spk accumulates a full [128, BS] row-block per j so each j
            # needs only one output DMA; the final DVE op is just one
            # b-half, keeping the tail short.
            spk_tiles = []
            for j in range(JC):
                spk = sbuf.tile([128, BS], mybir.dt.uint8, tag=f"spk{j}")
                spk_tiles.append(spk)
            for b in range(BC):
                for j in range(JC):
                    acc = psum.tile(
                        [128, NB], mybir.dt.float32, tag=f"acc{j}{b}", name=f"acc{j}{b}"
                    )
                    for k in range(KC):
                        nc.tensor.matmul(
                            acc,
                            wt_tiles[k][:, j * 128:(j + 1) * 128],
                            xt_tiles[k][b][:],
                            start=(k == 0),
                            stop=(k == KC - 1),
                        )
                    nc.vector.tensor_scalar(
                        spk_tiles[j][:, b * NB:(b + 1) * NB],
                        acc,
                        thr_t[:, j:j + 1],
                        None,
                        op0=mybir.AluOpType.is_gt,
                    )
                    if b == BC - 1:
                        rings[j % 2].dma_start(
                            out=outT[j * 128:(j + 1) * 128, :],
                            in_=spk_tiles[j][:],
                        )
    nc.compile()
    return nc


def _get_nc():
    if not _NC_CACHE:
        _NC_CACHE.append(_build_nc())
    return _NC_CACHE[0]


def _run(x, W_lin, b_lin, **spmd_kwargs):
    WT = np.ascontiguousarray(W_lin.T)
    thr = np.ascontiguousarray((np.float32(1.0) - b_lin).reshape(2, 128, 1))
    in_maps = []
    for c in range(NCORES):
        xTc = np.ascontiguousarray(x[c * BS:(c + 1) * BS].T)
        in_maps.append({"xT": xTc, "WT": WT, "thr": thr})
    return run_bass_kernel_spmd(
        _get_nc(), in_maps, core_ids=list(range(NCORES)), **spmd_kwargs
    )


def kernel(x, W_lin, b_lin, W_sel=None, b_sel=None):
    x = np.asarray(x, dtype=np.float32)
    W_lin = np.asarray(W_lin, dtype=np.float32)
    b_lin = np.asarray(b_lin, dtype=np.float32)

    res = _run(x, W_lin, b_lin)

    out = np.zeros((B_FULL, OUT_COLS), dtype=np.float32)
    for c in range(NCORES):
        out[c * BS:(c + 1) * BS, :N] = res.results[c]["outT"].T
    return out


# revision 23
# speedup vs baseline: 1.0898x; 1.0898x over previous
"""Trainium2 Bass kernel for nn_PolymorphicSNN.

Reference math:
    h        = x @ W_lin.T + b_lin                       # [B, N]
    reg_out  = heaviside(h - 1.0)            in {0, 1}   # [B, N]
    scores   = softmax(einsum(reg_out, W_sel) + b_sel)   # [B, P, M] (finite)
    spk_mode = heaviside(reg_out - 1.0)                  # == 0 exactly!
    mixed    = M * spk_mode * scores                     # == 0 exactly
    out      = concat([reg_out, mixed.reshape(B, -1)])   # [B, N*(1+P)]

Since reg_out in {0.0, 1.0}, (reg_out - 1.0 > 0) is identically False, so
spk_mode == 0 and the whole mixed block is exactly zero (scores are finite
softmax outputs, so 0 * scores == 0). The only device work needed is the
[B, N] spike map:  reg_out = (x @ W_lin.T > 1 - b_lin).

Strategy: pure data-parallel over 8 NeuronCores, batch-sharded (1024 rows
per core). Host pre-transposes the shard (xT [N, BS]) and the weight
(WT = W_lin.T) so the contraction dim lands on SBUF partitions; the bias
and threshold fold into a single per-partition compare:
    outT[j, b] = (W @ x^T)[j, b] > (1 - b_lin[j])
as fp32 PE matmuls + DVE is_gt (uint8 out). Input DMAs are split across
both HWDGE rings (sync + scalar) and by batch half so matmuls start as
soon as the first half lands; scratch warmup matmuls keep the PE HAM
clock-gate warm through the DMA window. Host assembles the full
[8192, 8448] float32 output with the zero block.
"""

import time

import numpy as np

import concourse.bacc as bacc
import concourse.mybir as mybir
from concourse.bass_utils import run_bass_kernel_spmd
from concourse.tile import TileContext

B_FULL = 8192
N = 256                     # features / neurons / modes
P_SEL = 32                  # polymorphic neuron count (dead on device)
NCORES = 8
BS = B_FULL // NCORES       # 1024 batch rows per core
OUT_COLS = N * (1 + P_SEL)  # 8448
NB = 512                    # matmul moving free dim (one f32 PSUM bank)
N_WARMUP = 3                # scratch matmuls to warm the PE clock gate
MEMSET_WARMUP = True        # zero the warmup scratch first (vs garbage)

_NC_CACHE = []


def _build_nc():
    nc = bacc.Bacc("TRN2", target_bir_lowering=False, debug=False)
    # pk0: [ WT rows 0:128 | xT rows 0:128 cols 0:512 | thr [128,2] ]
    # pk1: [ WT rows 128:256 | xT rows 128:256 cols 0:512 ]
    # One DMA per ring delivers everything the first matmul group needs.
    pk0 = nc.declare_dram_parameter("pk0", [128, N + NB + 2], mybir.dt.float32, isOutput=False)
    pk1 = nc.declare_dram_parameter("pk1", [128, N + NB], mybir.dt.float32, isOutput=False)
    xb1 = nc.declare_dram_parameter("xb1", [N, NB], mybir.dt.float32, isOutput=False)
    outT = nc.declare_dram_parameter("outT", [N, BS], mybir.dt.uint8, isOutput=True)

    KC = N // 128    # contraction chunks
    JC = N // 128    # output-neuron chunks
    BC = BS // NB    # batch chunks

    rings = [nc.sync, nc.scalar]  # the two HWDGE rings

    with TileContext(nc) as tc:
        with (
            tc.tile_pool(name="sbuf", bufs=1) as sbuf,
            tc.tile_pool(name="psum", bufs=1, space="PSUM") as psum,
        ):
            # --- input DMAs, split across both HWDGE rings -------------
            p0 = sbuf.tile([128, N + NB + 2], mybir.dt.float32, tag="p0")
            rings[0].dma_start(out=p0[:], in_=pk0[:, :])
            p1 = sbuf.tile([128, N + NB], mybir.dt.float32, tag="p1")
            rings[1].dma_start(out=p1[:], in_=pk1[:, :])
            wt_tiles = [p0[:, 0:N], p1[:, 0:N]]
            thr_t = p0[:, N + NB:N + NB + 2]
            xt_tiles = [[p0[:, N:N + NB], None], [p1[:, N:N + NB], None]]
            for k in range(KC):
                xt = sbuf.tile([128, NB], mybir.dt.float32, tag=f"xt{k}1")
                rings[k % 2].dma_start(
                    out=xt[:], in_=xb1[k * 128:(k + 1) * 128, :]
                )
                xt_tiles[k][1] = xt

            # --- PE warmup on zero scratch while inputs stream ---------
            # no deps on the DMA'd tiles, so these run during the DMA window
            wsrc = sbuf.tile([128, NB], mybir.dt.float32, tag="wsrc")
            if MEMSET_WARMUP:
                nc.vector.memset(wsrc[:], 0.0)
            wacc = psum.tile([128, NB], mybir.dt.float32, tag="wacc", name="wacc")
            for _ in range(N_WARMUP):
                nc.tensor.matmul(
                    wacc,
                    wsrc[:, 0:128],
                    wsrc[:],
                    start=True,
                    stop=True,
                    skip_group_check=True,
                )

            # --- matmul + threshold + store, b-half at a time ----------
            # spk accumulates a full [128, BS] row-block per j so each j
            # needs only one output DMA; the final DVE op is just one
            # b-half, keeping the tail short.
            spk_tiles = []
            for j in range(JC):
                spk = sbuf.tile([128, BS], mybir.dt.uint8, tag=f"spk{j}")
                spk_tiles.append(spk)
            for b in range(BC):
                for j in range(JC):
                    acc = psum.tile(
                        [128, NB], mybir.dt.float32, tag=f"acc{j}{b}", name=f"acc{j}{b}"
                    )
                    for k in range(KC):
                        nc.tensor.matmul(
                            acc,
                            wt_tiles[k][:, j * 128:(j + 1) * 128],
                            xt_tiles[k][b][:],
                            start=(k == 0),
                            stop=(k == KC - 1),
                        )
                    nc.vector.tensor_scalar(
                        spk_tiles[j][:, b * NB:(b + 1) * NB],
                        acc,
                        thr_t[:, j:j + 1],
                        None,
                        op0=mybir.AluOpType.is_gt,
                    )
                    if b == BC - 1:
                        rings[j % 2].dma_start(
                            out=outT[j * 128:(j + 1) * 128, :],
                            in_=spk_tiles[j][:],
                        )
    nc.compile()
    return nc


def _get_nc():
    if not _NC_CACHE:
        _NC_CACHE.append(_build_nc())
    return _NC_CACHE[0]


def _run(x, W_lin, b_lin, **spmd_kwargs):
    WT = W_lin.T  # [n, j]
    thr = (np.float32(1.0) - b_lin).reshape(2, 128).T  # [128, 2], thr[p,c]=1-b[c*128+p]
    in_maps = []
    for c in range(NCORES):
        xTc = x[c * BS:(c + 1) * BS].T  # [n, b]
        pk0 = np.ascontiguousarray(
            np.concatenate([WT[0:128, :], xTc[0:128, 0:NB], thr], axis=1)
        )
        pk1 = np.ascontiguousarray(
            np.concatenate([WT[128:256, :], xTc[128:256, 0:NB]], axis=1)
        )
        xb1 = np.ascontiguousarray(xTc[:, NB:BS])
        in_maps.append({"pk0": pk0, "pk1": pk1, "xb1": xb1})
    last_err = None
    for attempt in range(3):
        try:
            return run_bass_kernel_spmd(
                _get_nc(), in_maps, core_ids=list(range(NCORES)), **spmd_kwargs
            )
        except Exception as e:  # transient axon/NRT device errors
            last_err = e
            try:
                import jax

                jax.clear_caches()
            except Exception:
                pass
            time.sleep(2.0 * (attempt + 1))
    raise last_err


def kernel(x, W_lin, b_lin, W_sel=None, b_sel=None):
    x = np.asarray(x, dtype=np.float32)
    W_lin = np.asarray(W_lin, dtype=np.float32)
    b_lin = np.asarray(b_lin, dtype=np.float32)

    res = _run(x, W_lin, b_lin)

    out = np.zeros((B_FULL, OUT_COLS), dtype=np.float32)
    for c in range(NCORES):
        out[c * BS:(c + 1) * BS, :N] = res.results[c]["outT"].T
    return out


# revision 28
# speedup vs baseline: 1.1550x; 1.0599x over previous
"""Trainium2 Bass kernel for nn_PolymorphicSNN.

Reference math:
    h        = x @ W_lin.T + b_lin                       # [B, N]
    reg_out  = heaviside(h - 1.0)            in {0, 1}   # [B, N]
    scores   = softmax(einsum(reg_out, W_sel) + b_sel)   # [B, P, M] (finite)
    spk_mode = heaviside(reg_out - 1.0)                  # == 0 exactly!
    mixed    = M * spk_mode * scores                     # == 0 exactly
    out      = concat([reg_out, mixed.reshape(B, -1)])   # [B, N*(1+P)]

Since reg_out in {0.0, 1.0}, (reg_out - 1.0 > 0) is identically False, so
spk_mode == 0 and the whole mixed block is exactly zero (scores are finite
softmax outputs, so 0 * scores == 0). The only device work needed is the
[B, N] spike map:  reg_out = (x @ W_lin.T > 1 - b_lin).

Strategy: pure data-parallel over 8 NeuronCores, batch-sharded (1024 rows
per core). Host pre-transposes the shard (xT [N, BS]) and the weight
(WT = W_lin.T) so the contraction dim lands on SBUF partitions; the bias
and threshold fold into a single per-partition compare:
    outT[j, b] = (W @ x^T)[j, b] > (1 - b_lin[j])
as fp32 PE matmuls + DVE is_gt (uint8 out). Input DMAs are split across
both HWDGE rings (sync + scalar) and by batch half so matmuls start as
soon as the first half lands; scratch warmup matmuls keep the PE HAM
clock-gate warm through the DMA window. Host assembles the full
[8192, 8448] float32 output with the zero block.
"""

import time

import numpy as np

import concourse.bacc as bacc
import concourse.mybir as mybir
from concourse.bass_utils import run_bass_kernel_spmd
from concourse.tile import TileContext

B_FULL = 8192
N = 256                     # features / neurons / modes
P_SEL = 32                  # polymorphic neuron count (dead on device)
NCORES = 8
BS = B_FULL // NCORES       # 1024 batch rows per core
OUT_COLS = N * (1 + P_SEL)  # 8448
NB = 512                    # matmul moving free dim (one f32 PSUM bank)
N_WARMUP = 2                # scratch matmuls to warm the PE clock gate
MEMSET_WARMUP = True        # zero the warmup scratch first (vs garbage)

_NC_CACHE = []


def _build_nc():
    nc = bacc.Bacc("TRN2", target_bir_lowering=False, debug=False)
    # pk0: [ WT rows 0:128 | xT rows 0:128 cols 0:512 | thr [128,2] ]
    # pk1: [ WT rows 128:256 | xT rows 128:256 cols 0:512 ]
    # One DMA per ring delivers everything the first matmul group needs.
    pk0 = nc.declare_dram_parameter("pk0", [128, N + NB + 2], mybir.dt.float32, isOutput=False)
    pk1 = nc.declare_dram_parameter("pk1", [128, N + NB], mybir.dt.float32, isOutput=False)
    xb1 = nc.declare_dram_parameter("xb1", [N, NB], mybir.dt.float32, isOutput=False)
    outT = nc.declare_dram_parameter("outT", [N, BS], mybir.dt.uint8, isOutput=True)

    KC = N // 128    # contraction chunks
    JC = N // 128    # output-neuron chunks
    BC = BS // NB    # batch chunks

    rings = [nc.sync, nc.scalar]  # the two HWDGE rings

    with TileContext(nc) as tc:
        with (
            tc.tile_pool(name="sbuf", bufs=1) as sbuf,
            tc.tile_pool(name="psum", bufs=1, space="PSUM") as psum,
        ):
            # --- input DMAs, split across both HWDGE rings -------------
            p0 = sbuf.tile([128, N + NB + 2], mybir.dt.float32, tag="p0")
            rings[0].dma_start(out=p0[:], in_=pk0[:, :])
            p1 = sbuf.tile([128, N + NB], mybir.dt.float32, tag="p1")
            rings[1].dma_start(out=p1[:], in_=pk1[:, :])
            wt_tiles = [p0[:, 0:N], p1[:, 0:N]]
            thr_t = p0[:, N + NB:N + NB + 2]
            xt_tiles = [[p0[:, N:N + NB], None], [p1[:, N:N + NB], None]]
            for k in range(KC):
                xt = sbuf.tile([128, NB], mybir.dt.float32, tag=f"xt{k}1")
                rings[k % 2].dma_start(
                    out=xt[:], in_=xb1[k * 128:(k + 1) * 128, :]
                )
                xt_tiles[k][1] = xt

            # --- PE warmup on zero scratch while inputs stream ---------
            # no deps on the DMA'd tiles, so these run during the DMA window
            wsrc = sbuf.tile([128, NB], mybir.dt.float32, tag="wsrc")
            if MEMSET_WARMUP:
                nc.vector.memset(wsrc[:], 0.0)
            wacc = psum.tile([128, NB], mybir.dt.float32, tag="wacc", name="wacc")
            for _ in range(N_WARMUP):
                nc.tensor.matmul(
                    wacc,
                    wsrc[:, 0:128],
                    wsrc[:],
                    start=True,
                    stop=True,
                    skip_group_check=True,
                )

            # --- matmul + threshold + store, b-half at a time ----------
            # spk accumulates a full [128, BS] row-block per j so each j
            # needs only one output DMA; the final DVE op is just one
            # b-half, keeping the tail short.
            spk_tiles = []
            for j in range(JC):
                spk = sbuf.tile([128, BS], mybir.dt.uint8, tag=f"spk{j}")
                spk_tiles.append(spk)
            for b in range(BC):
                for j in range(JC):
                    acc = psum.tile(
                        [128, NB], mybir.dt.float32, tag=f"acc{j}{b}", name=f"acc{j}{b}"
                    )
                    for k in range(KC):
                        nc.tensor.matmul(
                            acc,
                            wt_tiles[k][:, j * 128:(j + 1) * 128],
                            xt_tiles[k][b][:],
                            start=(k == 0),
                            stop=(k == KC - 1),
                        )
                    nc.vector.tensor_scalar(
                        spk_tiles[j][:, b * NB:(b + 1) * NB],
                        acc,
                        thr_t[:, j:j + 1],
                        None,
                        op0=mybir.AluOpType.is_gt,
                    )
                    if b == BC - 1:
                        rings[j % 2].dma_start(
                            out=outT[j * 128:(j + 1) * 128, :],
                            in_=spk_tiles[j][:],
                        )
    nc.compile()
    return nc


def _get_nc():
    if not _NC_CACHE:
        _NC_CACHE.append(_build_nc())
    return _NC_CACHE[0]


def _run(x, W_lin, b_lin, **spmd_kwargs):
    WT = W_lin.T  # [n, j]
    thr = (np.float32(1.0) - b_lin).reshape(2, 128).T  # [128, 2], thr[p,c]=1-b[c*128+p]
    in_maps = []
    for c in range(NCORES):
        xTc = x[c * BS:(c + 1) * BS].T  # [n, b]
        pk0 = np.ascontiguousarray(
            np.concatenate([WT[0:128, :], xTc[0:128, 0:NB], thr], axis=1)
        )
        pk1 = np.ascontiguousarray(
            np.concatenate([WT[128:256, :], xTc[128:256, 0:NB]], axis=1)
        )
        xb1 = np.ascontiguousarray(xTc[:, NB:BS])
        in_maps.append({"pk0": pk0, "pk1": pk1, "xb1": xb1})
    last_err = None
    for attempt in range(3):
        try:
            return run_bass_kernel_spmd(
                _get_nc(), in_maps, core_ids=list(range(NCORES)), **spmd_kwargs
            )
        except Exception as e:  # transient axon/NRT device errors
            last_err = e
            try:
                import jax

                jax.clear_caches()
            except Exception:
                pass
            time.sleep(2.0 * (attempt + 1))
    raise last_err


def kernel(x, W_lin, b_lin, W_sel=None, b_sel=None):
    x = np.asarray(x, dtype=np.float32)
    W_lin = np.asarray(W_lin, dtype=np.float32)
    b_lin = np.asarray(b_lin, dtype=np.float32)

    res = _run(x, W_lin, b_lin)

    out = np.zeros((B_FULL, OUT_COLS), dtype=np.float32)
    for c in range(NCORES):
        out[c * BS:(c + 1) * BS, :N] = res.results[c]["outT"].T
    return out
